# revision 58
# baseline (speedup 1.0000x reference)
"""CSPN (convolutional spatial propagation) Trainium2 kernel, v2.

Full inputs:  guidance [8, 8, 512, 512] f32, x [8, 1, 512, 512] f32.
Sharding: data-parallel over batch -- core b gets batch element b.

Per-core algorithm (all SBUF-resident, fp16 storage / fp32 PSUM accumulate):
  band layout: partition p holds image rows 4p..4p+3 as "slots" in the free
  dim (pitch 520 fp16; storage col c in [0,513] <-> image col c-1, cols 0 and
  513 are zero pads).  h additionally has halo slots 0 (row 4p-1, copied from
  partition p-1) and 5 (row 4p+4) so ALL stencil reads are free-dim reads.

  graw/gt/pr are TAP-indexed [128, 9, NS, PW] (tap k = (di+1)*3+(dj+1), k=4
  is the center); guidance channel ch lands at tap ch (ch<4) or ch+1, so the
  dj-grouped tap sets {0,3,6}, {1,7}, {2,5,8} are uniform-stride slices.

  setup: guidance DMA'd f32->f16 in two 4-channel chunks per slot (Pool
  SWDGE); A = sum|g_k| via DVE bitwise-and abs (4x tensor-scalar mode) + PE
  identity-matmul PSUM accumulation; r = Exp(-Log(A + 1e-8)) on ACT, written
  fp16; gt_k = graw_k * r in 3 dj-grouped DVE muls per slot (stored column-
  pre-shifted by -dj so iteration tap reads are pure free-dim offsets).
  Center gate gt_4 = 1 - sum_k gt_k of the fp16-rounded weights (PE sum +
  ACT 1-x eviction) keeps row sums exact.  h is stored scaled by RESCALE^t
  (the operator grows ~2.45x/iter and would overflow fp16).

  iteration (x24): 36 product planes/iter.  DVE takes 29 of them in 8
  instructions per ITER_PRODS -- 3-tap di-groups plus two dependency-
  compatible 6-tap merges -- emitted latest-dependency-last so the DVE
  stream never stalls on the previous iteration's evictions (the
  steady-state period equals DVE's busy time; DVE is gapless end to end).
  Pool (gpsimd) takes slot 0's taps 0-6 (7 planes, 3 ops; slot-0 products
  double-buffer between pr slot columns 0 and 4 so no WAR waits).  PE
  consumes every chunk as it lands via one global pass stream mirroring
  DVE/Pool emission (per-bank psum chains interleave); ACT evicts
  PSUM -> new h (fp16) with the 0.5 rescale folded in -- except iteration
  23, which evicts fp32 at true magnitude (scale 2^23) straight into the
  output staging buffer, from which the sync engine HWDGE-DMAs each slot
  to DRAM as it lands.  Sync engine also refreshes the two halo rows per
  iteration (skipped after the last); iteration 23's slot-0 taps 3-5 run
  on DVE so the tail never waits for the Pool queue.

  Tap 8 of slot 0 is additionally column-split (Pool cols 0:258, DVE
  258:514) to use Pool's residual per-iteration idle time.

  cost-model makespan: 233.1 us vs 260.1 us for the previous version
  (input-DMA floor ~26 us + 24 x ~8.3 us DVE-bound iterations + ~6 us
  tail).
"""

import sys

sys.path.insert(0, "/opt/trn_rl_repo")

import numpy as np

import concourse.bass as bass
from concourse import mybir
from concourse.bass_utils import run_bass_kernel_spmd
from concourse.alu_op_type import AluOpType

F16 = mybir.dt.float16
F32 = mybir.dt.float32
U16 = mybir.dt.uint16
AF = mybir.ActivationFunctionType

N_CORES = 8
H, W = 512, 512
NS = 4            # data row-slots per partition
PW = 520          # fp16 elements per row slot (514 used + pad)
NITER = 24
# tap k = (di+1)*3 + (dj+1); k=4 is the center gate
OFFS = [(k // 3 - 1, k % 3 - 1) for k in range(9)]
MORD = [2, 1, 3, 0]   # eviction order (s_pe / s_ev count order)
SORD = MORD           # setup slot order matches iteration-0 consumption
# within-slot product-group emission order on DVE: latest-dependency last
# (deps per group g of slot sig: prev-iter evict of image slot sig+g-1)
# DVE iteration-product emission: (sig, k0, nk, dep).  dep ('ev', s) = the
# previous iteration's eviction of image slot s (the h slot this chunk
# reads); ('haloB',) = the bottom halo DMA.  Ordered so every chunk's dep
# has landed by the time DVE reaches it (latest deps last), with
# dependency-compatible group pairs merged into 6-tap instructions.
ITER_PRODS = [
    (2, 3, 3, ('ev', 2), None),
    (2, 0, 3, ('ev', 1), None),
    (2, 6, 3, ('ev', 3), None),
    (3, 0, 6, ('ev', 3), None),
    (1, 3, 6, ('ev', 1), None),
    (1, 0, 3, ('ev', 0), None),
    (3, 6, 3, ('haloB',), None),
    (0, 7, 1, ('ev', 1), None),
    # tap 8 of slot 0 is column-split with Pool (Pool does cols 0:258)
    (0, 8, 1, ('ev', 1), (258, 514)),
]
# dj-grouped tap sets for the setup scale-muls (uniform tap stride)
DJ_TAPS = [(0, 3, 3), (1, 2, 6), (2, 3, 3)]  # (first_tap, count, stride)
RESCALE = 0.5


def build_program(niter=NITER):
    nc = bass.Bass("TRN2", target_bir_lowering=False, debug=False)

    g_dram = nc.dram_tensor("guidance", [8, H, W], F32, kind="ExternalInput")
    x_dram = nc.dram_tensor("x", [1, H, W], F32, kind="ExternalInput")
    o_dram = nc.dram_tensor("out", [1, H, W], F32, kind="ExternalOutput")

    h0 = nc.alloc_sbuf_tensor("h0", [128, 6, PW], F16)
    h1 = nc.alloc_sbuf_tensor("h1", [128, 6, PW], F16)
    gt = nc.alloc_sbuf_tensor("gt", [128, 9, NS, PW], F16)
    # pr has a 5th slot column: slot-0 products double-buffer between slot
    # indices 0 (even t) and 4 (odd t) so their producers never carry a WAR
    # wait against PE's slot-0 pass of the previous iteration.
    pr = nc.alloc_sbuf_tensor("pr", [128, 9, NS + 1, PW], F16)
    graw = nc.alloc_sbuf_tensor("graw", [128, 9, NS, PW], F16)
    gabs = nc.alloc_sbuf_tensor("gabs", [128, 2, 8, PW], F16)
    asb = nc.alloc_sbuf_tensor("asb", [128, NS, PW], F32)   # log(A); out stage
    rr = nc.alloc_sbuf_tensor("rr", [128, NS, PW], F16)     # r = 1/A
    ident = nc.alloc_sbuf_tensor("ident", [128, 128], F16)
    c_eps = nc.alloc_sbuf_tensor("c_eps", [128, 1], F32)

    psum = [nc.alloc_psum_tensor(f"pg{g}s{s}", [128, W], F32)
            for g in range(2) for s in range(NS)]

    def pg(g, s):
        return psum[g * NS + s].ap()

    hb = [h0, h1]

    s_hz = nc.alloc_semaphore("s_hz")      # DVE memsets done
    s_id = nc.alloc_semaphore("s_id")      # identity built
    s_x = nc.alloc_semaphore("s_x")        # x DMA (+16)
    s_gs = [nc.alloc_semaphore(f"s_g{i}") for i in range(NS)]
    s_abs = nc.alloc_semaphore("s_abs")    # DVE abs (2 per slot)
    s_apex = nc.alloc_semaphore("s_apex")  # PE A-sum per slot
    s_rq = nc.alloc_semaphore("s_rq")      # ACT ln per slot
    s_rexp = nc.alloc_semaphore("s_rexp")  # ACT exp -> rr per slot
    s_gtd = nc.alloc_semaphore("s_gtd")    # DVE scale-muls (3/slot)
    s_cpe = nc.alloc_semaphore("s_cpe")    # PE center sum per slot
    s_cev = nc.alloc_semaphore("s_cev")    # ACT center evict per slot
    s_mul = nc.alloc_semaphore("s_mul")    # DVE iter products (4/iter)
    s_mulp = nc.alloc_semaphore("s_mulp")  # Pool iter products (3/iter)
    s_pe = nc.alloc_semaphore("s_pe")      # PE tap-sum per slot (4/iter)
    s_ev = nc.alloc_semaphore("s_ev")      # ACT evict per slot (4/iter)
    s_haloA = nc.alloc_semaphore("s_haloA")  # top-halo DMA (+16/iter)
    s_haloB = nc.alloc_semaphore("s_haloB")  # bot-halo DMA (+16/iter)
    s_out = nc.alloc_semaphore("s_out")    # output DMAs (+16 each)

    posM = {sg: i for i, sg in enumerate(MORD)}

    def s0pr(t):
        """pr slot index for slot-0 products of iteration t (double-buffer)."""
        return 0 if t % 2 == 0 else NS

    def prod_taps(eng, t, sig, k0, nk, c0=0, ce=514):
        """Product planes for taps k0..k0+nk-1 of slot sig.  nk <= 3 reads a
        single h slot; nk == 6 spans two whole di-groups (two h slots).
        c0:ce restricts the column range (for planes split across engines)."""
        hcur = hb[t % 2]
        hs = sig + k0 // 3
        ps = s0pr(t) if sig == 0 else sig
        if nk <= 3 and (c0, ce) != (0, 514):
            return eng.tensor_tensor(
                out=pr.ap()[:, k0:k0 + nk, ps, c0:ce],
                in0=gt.ap()[:, k0:k0 + nk, sig, c0:ce],
                in1=hcur.ap()[:, hs:hs + 1, c0:ce]
                    .broadcast_to([128, nk, ce - c0]),
                op=AluOpType.mult,
            )
        if nk == 6:
            out = (pr.ap()[:, k0:k0 + 6, ps, 0:514]
                   .rearrange("p (a b) c -> p a b c", a=2))
            in0 = (gt.ap()[:, k0:k0 + 6, sig, 0:514]
                   .rearrange("p (a b) c -> p a b c", a=2))
            in1 = (hcur.ap()[:, hs:hs + 2, 0:514]
                   .unsqueeze(2).broadcast_to([128, 2, 3, 514]))
            return eng.tensor_tensor(out=out, in0=in0, in1=in1,
                                     op=AluOpType.mult)
        return eng.tensor_tensor(
            out=pr.ap()[:, k0:k0 + nk, ps, 0:514],
            in0=gt.ap()[:, k0:k0 + nk, sig, 0:514],
            in1=hcur.ap()[:, hs:hs + 1, 0:514].broadcast_to([128, nk, 514]),
            op=AluOpType.mult,
        )

    with nc.Block() as block:

        # ---------------- GPSIMD (Pool): ident, input DMAs, slot-0 products -
        @block.gpsimd
        def _(gp):
            gp.wait_ge(s_hz, 2)
            gp.affine_select(
                out=ident.ap(), in_=ident.ap(),
                compare_op=AluOpType.not_equal, fill=1.0, base=0,
                pattern=[[-1, 128]], channel_multiplier=1,
            ).then_inc(s_id, 1)
            # guidance: two 4-channel cast DMAs per row-slot, SORD order.
            # channels 0-3 -> taps 0-3, channels 4-7 -> taps 5-8.
            for s in SORD:
                for half, tap0 in ((0, 0), (1, 5)):
                    g_in = bass.AP(g_dram, 4 * half * H * W + s * W,
                                   [[4 * W, 128], [H * W, 4], [1, W]])
                    gp.dma_start(
                        graw.ap()[:, tap0:tap0 + 4, s, 1:513], g_in
                    ).then_inc(s_gs[s], 16)
            # x -> h0 data slots (cast f32->f16)
            gp.wait_ge(s_hz, 1)
            x_in = bass.AP(x_dram, 0, [[4 * W, 128], [W, NS], [1, W]])
            gp.dma_start(h0.ap()[:, 1:5, 1:513], x_in).then_inc(s_x, 16)
            # iteration products for slot 0, taps 0..6 (3 instructions),
            # latest-dependency last: tap 6 (prev evict of image slot 1),
            # taps 0-2 (top halo), taps 3-5 (prev evict of image slot 0).
            for t in range(niter):
                if t == 0:
                    gp.wait_ge(s_gtd, 12)
                    gp.wait_ge(s_x, 16)
                else:
                    gp.wait_ge(s_ev, 4 * (t - 1) + 2)
                prod_taps(gp, t, 0, 6, 1).then_inc(s_mulp, 1)
                # tap 8 columns 0:258 (DVE computes 258:514); early dep
                prod_taps(gp, t, 0, 8, 1, 0, 258).then_inc(s_mulp, 1)
                gp.wait_ge(s_haloA, 16 * (t + 1))
                prod_taps(gp, t, 0, 0, 3).then_inc(s_mulp, 1)
                if t == niter - 1:
                    continue  # final-iteration taps 3-5 run on DVE (tail)
                if t == 0:
                    gp.wait_ge(s_cev, 4)
                else:
                    gp.wait_ge(s_ev, 4 * (t - 1) + 4)
                prod_taps(gp, t, 0, 3, 3).then_inc(s_mulp, 1)

        # ---------------- DVE: memsets, abs, scale-muls, main products ------
        @block.vector
        def _(v):
            v.memset(h0.ap(), 0.0).then_inc(s_hz, 1)
            v.memset(ident.ap(), 0.0).then_inc(s_hz, 1)
            v.memset(c_eps.ap(), 1e-8).then_inc(s_hz, 1)
            v.memset(gt.ap()[:, :, :, 0:2], 0.0).then_inc(s_hz, 1)
            v.memset(gt.ap()[:, :, :, 512:520], 0.0).then_inc(s_hz, 1)
            # abs per slot as DMA halves land (SORD order)
            for p, s in enumerate(SORD):
                v.wait_ge(s_gs[s], 16)
                if p >= 2:
                    v.wait_ge(s_apex, p - 1)  # gabs[p%2] free again
                v.tensor_scalar(
                    out=gabs.ap()[:, p % 2, 0:4, 1:513].bitcast(U16),
                    in0=graw.ap()[:, 0:4, s, 1:513].bitcast(U16),
                    scalar1=0x7FFF, scalar2=None, op0=AluOpType.bitwise_and,
                ).then_inc(s_abs, 1)
                v.wait_ge(s_gs[s], 32)
                v.tensor_scalar(
                    out=gabs.ap()[:, p % 2, 4:8, 1:513].bitcast(U16),
                    in0=graw.ap()[:, 5:9, s, 1:513].bitcast(U16),
                    scalar1=0x7FFF, scalar2=None, op0=AluOpType.bitwise_and,
                ).then_inc(s_abs, 1)
            # scale-muls gt_k = graw_k * r, dj-grouped (column-pre-shift -dj)
            for p, s in enumerate(SORD):
                v.wait_ge(s_rexp, p + 1)
                for dj, (k0, nk, kstep) in zip((-1, 0, 1), DJ_TAPS):
                    kend = k0 + (nk - 1) * kstep + 1
                    v.tensor_tensor(
                        out=gt.ap()[:, k0:kend:kstep, s, 1 + dj:513 + dj],
                        in0=graw.ap()[:, k0:kend:kstep, s, 1:513],
                        in1=rr.ap()[:, s:s + 1, 1:513]
                            .broadcast_to([128, nk, 512]),
                        op=AluOpType.mult,
                    ).then_inc(s_gtd, 1)
            # iterations: per-group products (3 taps), latest-dependency-last
            # ordering within each slot so DVE never stalls on a late evict.
            # slots 2,1,3 fully on DVE (3 groups each); slot 0: taps 7,8 only.
            for t in range(niter):
                for sig, k0, nk, dep, cols in ITER_PRODS:
                    if t == 0:
                        if k0 <= 4 < k0 + nk:
                            v.wait_ge(s_cev, posM[sig] + 1)
                        v.wait_ge(s_x, 16)
                        if dep[0] == 'haloB':
                            v.wait_ge(s_haloB, 16)
                    elif dep[0] == 'haloB':
                        v.wait_ge(s_haloB, 16 * (t + 1))
                    else:
                        v.wait_ge(s_ev, 4 * (t - 1) + posM[dep[1]] + 1)
                    c0, ce = cols if cols else (0, 514)
                    prod_taps(v, t, sig, k0, nk, c0, ce).then_inc(s_mul, 1)
                if t == niter - 1:
                    # final iteration: slot-0 taps 3-5 on DVE so the tail
                    # doesn't wait for the Pool queue to drain
                    v.wait_ge(s_ev, 4 * (t - 1) + 4)
                    prod_taps(v, t, 0, 3, 3).then_inc(s_mul, 1)

        # ---------------- PE: setup sums + iteration tap-sums ----------------
        @block.tensor
        def _(pe):
            pe.wait_ge(s_id, 1)
            # A = sum |g| into psum group 0 (8 planes per slot)
            for p, s in enumerate(SORD):
                pe.wait_ge(s_abs, 2 * p + 1)
                for j in range(8):
                    if j == 4:
                        pe.wait_ge(s_abs, 2 * p + 2)
                    inst = pe.matmul(
                        pg(0, s)[:, 0:512], ident.ap(),
                        gabs.ap()[:, p % 2, j, 1:513],
                        start=(j == 0), stop=(j == 7),
                    )
                    if j == 7:
                        inst.then_inc(s_apex, 1)
            # center gate: sum of fp16-rounded weights into psum group 1
            for p, s in enumerate(SORD):
                pe.wait_ge(s_gtd, 3 * (p + 1))
                done = 0
                for k in range(9):
                    if k == 4:
                        continue
                    dj = OFFS[k][1]
                    inst = pe.matmul(
                        pg(1, s)[:, 0:512], ident.ap(),
                        gt.ap()[:, k, s, 1 + dj:513 + dj],
                        start=(done == 0), stop=(done == 7),
                    )
                    done += 1
                    if done == 8:
                        inst.then_inc(s_cpe, 1)
            # iterations: one global chunk stream mirroring DVE/Pool emission
            # order so PE consumes every chunk as it lands; psum chains of
            # different slots interleave freely (separate banks).  s_pe
            # increments land in MORD order (2,1,3,0).  DVE s_mul counts per
            # iteration follow ITER_PRODS (1..8); Pool s_mulp: tap 6 = 1,
            # taps 0-2 = 2, taps 3-5 = 3.
            nip = len(ITER_PRODS)
            pe_stream = ([(sig, tuple(range(k0, k0 + nk)),
                           [(s_mul, i + 1, nip)])
                          for i, (sig, k0, nk, _, cols)
                          in enumerate(ITER_PRODS) if cols is None]
                         + [(0, (6,), [(s_mulp, 1, 4)]),
                            (0, (0, 1, 2), [(s_mulp, 3, 4)]),
                            (0, (8,), [(s_mul, nip, nip), (s_mulp, 2, 4)]),
                            (0, (3, 4, 5), [(s_mulp, 4, 4)])])
            # per-slot first/last tap in stream order (for start/stop flags)
            first_k = {}
            last_k = {}
            for sig, ks, _ in pe_stream:
                for k in ks:
                    first_k.setdefault(sig, k)
                    last_k[sig] = k
            for t in range(niter):
                for sig, ks, wlist in pe_stream:
                    for sem, cnt, per in wlist:
                        if t == niter - 1 and sem is s_mulp and cnt == 4:
                            # taps 3-5 came from DVE in the final iteration
                            pe.wait_ge(s_mul, nip * t + nip + 1)
                        else:
                            pe.wait_ge(sem, per * t + cnt)
                    ps = s0pr(t) if sig == 0 else sig
                    for k in ks:
                        dj = OFFS[k][1]
                        inst = pe.matmul(
                            pg(t % 2, sig)[:, 0:512], ident.ap(),
                            pr.ap()[:, k, ps, 1 + dj:513 + dj],
                            start=(k == first_k[sig]), stop=(k == last_k[sig]),
                        )
                        if k == last_k[sig]:
                            inst.then_inc(s_pe, 1)

        # ---------------- ACT: memset h1, r = exp(-log(A+eps)), evictions ----
        @block.scalar
        def _(sc):
            sc.memzero(h1.ap())
            sc.wait_ge(s_hz, 3)  # c_eps ready
            for p, s in enumerate(SORD):
                sc.wait_ge(s_apex, p + 1)
                sc.activation(
                    asb.ap()[:, s, 1:513], pg(0, s)[:, 0:512], AF.Ln,
                    bias=c_eps.ap(),
                ).then_inc(s_rq, 1)
                sc.wait_ge(s_rq, p + 1)
                sc.activation(
                    rr.ap()[:, s, 1:513], asb.ap()[:, s, 1:513], AF.Exp,
                    scale=-1.0,
                ).then_inc(s_rexp, 1)
            for p, s in enumerate(SORD):
                sc.wait_ge(s_cpe, p + 1)
                sc.activation(
                    gt.ap()[:, 4, s, 1:513], pg(1, s)[:, 0:512], AF.Identity,
                    bias=1.0, scale=-1.0,
                ).then_inc(s_cev, 1)
            for t in range(niter):
                hnext = hb[(t + 1) % 2]
                for pos, sig in enumerate(MORD):
                    sc.wait_ge(s_pe, 4 * t + pos + 1)
                    if t == niter - 1:
                        # final evict: fp32 at true magnitude, into out stage
                        sc.activation(
                            asb.ap()[:, sig, 1:513],
                            pg(t % 2, sig)[:, 0:512], AF.Copy,
                            scale=float((1.0 / RESCALE) ** (niter - 1)),
                        ).then_inc(s_ev, 1)
                    else:
                        sc.activation(
                            hnext.ap()[:, sig + 1, 1:513],
                            pg(t % 2, sig)[:, 0:512], AF.Copy,
                            scale=RESCALE,
                        ).then_inc(s_ev, 1)

        # ---------------- SYNC: halo DMAs (HWDGE) + output DMAs --------------
        @block.sync
        def _(sy):
            sy.wait_ge(s_x, 16)
            sy.wait_ge(s_hz, 1)
            sy.dma_start(h0.ap()[1:128, 0, 0:514],
                         h0.ap()[0:127, 4, 0:514]).then_inc(s_haloA, 16)
            sy.dma_start(h0.ap()[0:127, 5, 0:514],
                         h0.ap()[1:128, 1, 0:514]).then_inc(s_haloB, 16)
            for t in range(niter - 1):
                hnext = hb[(t + 1) % 2]
                sy.wait_ge(s_ev, 4 * t + posM[3] + 1)
                sy.wait_ge(s_haloA, 16 * (t + 1))
                sy.dma_start(hnext.ap()[1:128, 0, 0:514],
                             hnext.ap()[0:127, 4, 0:514]).then_inc(s_haloA, 16)
                sy.wait_ge(s_ev, 4 * t + posM[0] + 1)
                sy.wait_ge(s_haloB, 16 * (t + 1))
                sy.dma_start(hnext.ap()[0:127, 5, 0:514],
                             hnext.ap()[1:128, 1, 0:514]).then_inc(s_haloB, 16)
            # output: one DMA per slot as its final eviction lands
            for pos, sig in enumerate(MORD):
                sy.wait_ge(s_ev, 4 * (niter - 1) + pos + 1)
                o_out = bass.AP(o_dram, sig * W, [[4 * W, 128], [1, W]])
                sy.dma_start(o_out, asb.ap()[:, sig, 1:513]).then_inc(s_out, 16)
            sy.wait_ge(s_out, 64)

    return nc


_NC_CACHE = {}


def kernel(guidance: np.ndarray, x: np.ndarray) -> np.ndarray:
    """guidance [8,8,512,512] f32, x [8,1,512,512] f32 -> [8,1,512,512] f32."""
    guidance = np.ascontiguousarray(np.asarray(guidance, dtype=np.float32))
    x = np.ascontiguousarray(np.asarray(x, dtype=np.float32))
    if "nc" not in _NC_CACHE:
        _NC_CACHE["nc"] = build_program()
    nc = _NC_CACHE["nc"]
    in_maps = [
        {"guidance": guidance[b], "x": x[b].reshape(1, H, W)}
        for b in range(N_CORES)
    ]
    res = run_bass_kernel_spmd(nc, in_maps, core_ids=list(range(N_CORES)))
    out = np.stack([res.results[b]["out"] for b in range(N_CORES)], axis=0)
    return out.astype(np.float32)
